# revision 20
# baseline (speedup 1.0000x reference)
"""Batched DWI 3D conv as implicit GEMM on 8 TRN2 NeuronCores.

Problem: x [8, 12, 12, 32, 32, 32] f32, W [32, 12, 12, 3, 3, 3] f32
         -> out [8, 32, 30, 30, 30] f32  (VALID 3D conv, c_in = 144)

Strategy (data-parallel over batch, one batch element per core):
  - x viewed as [144, 32, 32, 32] in SBUF; a kernel offset (dx, dy, dz)
    is a pure free-dim shift, so the conv is a chain of shifted matmuls
    accumulated in PSUM: out[co, n] += W_d^T @ x[:, n + shift(d)].
  - c_out = 32 fills only 1/4 of the PE array columns, so 4 col-tiled
    matmuls run concurrently (tile_position=(0, 32j)), each computing a
    different output chunk into its own 32-partition PSUM slice. Measured:
    a 4-matmul phase = ~35 ns serialized issue + ~168 ns concurrent
    stream (N=450), so total PE time ~ #phases * 203 ns.
  - c_in = 144 = 128 + 16. Body: 27 ctile phases (K=128, AP shifts).
    Tail: the leftover 16 channels x 27 taps = 432 rows are packed as a
    host-built [128, 32^3] fp8 tile of 8 shifted copies (blocks S below);
    5 phase translates T cover all 27 taps (proved minimal: no 4-phase
    cover exists, even with flat-space translates). Tail data + weights
    are fp8 e4m3: adds ~1.3e-2 rel err (16/144 of the energy, sqrt(2) for
    both operands) - under the 2e-2 gate; halves tail DMA to 4 MB.
  - 32 phases/bank x 15 banks. Deferred tails: group g's 5 tail phases
    execute inside group g+1's body window (after its first 9 ctiles),
    so early groups never stall on tail DMA; g's PSUM banks stay open
    (no stop) until its tail finishes, then staged (bf16) and stored.
  - Output stored as bf16 (halves store traffic; +1e-3 rel err).
  - DMAs sized >=8KB per-partition records, >=128 partitions; issue
    order = consumption order. Dummy matmuls warm the PE clock (HAM)
    during the initial load window. All DMAs on the sync HWDGE ring.
"""

import numpy as np
import ml_dtypes

import concourse.bass as bass
import concourse.bacc as bacc
import concourse.mybir as mybir
import concourse.tile as tile
from concourse.bass_utils import run_bass_kernel_spmd

BF16 = mybir.dt.bfloat16
FP8 = mybir.dt.float8e4
F32 = mybir.dt.float32

N_CORES = 8
CIN = 144
COUT = 32
DIM = 32
ODIM = 30
NCH = 450  # one chunk = 15 y-rows x 30 z
NFLAT = DIM * DIM * DIM
CHUNKS = [(ox, h) for ox in range(ODIM) for h in (0, 1)]  # 60 chunks
# chunks per group; adjacent groups' bank counts must sum to <= 8 PSUM banks
GROUP_SIZES = [4, 8, 4, 16, 16, 8, 4]
# x body loaded in plane slabs sized to stay ahead of PE consumption
XSLABS = [(0, 1), (1, 2), (2, 4), (4, 8), (8, 16), (16, 24), (24, 32)]

# tail cover: tap (dx,dy,dz) = S[b] + T[i]; 8 baked block shifts, 5 phases
T_PHASES = [(0, 0, 0), (0, 0, 1), (0, 0, 2), (0, 1, 0), (1, 0, 0)]
S_BLOCKS = [(0, 0, 0), (0, 1, 0), (0, 2, 0), (1, 0, 0),
            (1, 1, 1), (1, 1, 2), (2, 0, 0), (2, 2, 0)]
NTAIL = len(T_PHASES)
N_CTILES = 27 + NTAIL


def _tail_assign():
    """tap -> (block, phase), first-match; every tap covered exactly once."""
    assign = {}
    for i, t in enumerate(T_PHASES):
        for b, s in enumerate(S_BLOCKS):
            tap = (s[0] + t[0], s[1] + t[1], s[2] + t[2])
            if all(0 <= v <= 2 for v in tap) and tap not in assign:
                assign[tap] = (b, i)
    assert len(assign) == 27, len(assign)
    return assign


_CACHE = {}


def build_nc():
    nc = bacc.Bacc(None, target_bir_lowering=False)
    xin = nc.dram_tensor("x", [128, DIM, DIM, DIM], BF16, kind="ExternalInput")
    xt_d = nc.dram_tensor("xt", [128, DIM, DIM, DIM], FP8, kind="ExternalInput")
    wf_d = nc.dram_tensor("wf", [128, 27, COUT], BF16, kind="ExternalInput")
    wt_d = nc.dram_tensor("wt", [128, NTAIL, COUT], FP8, kind="ExternalInput")
    n_banks_total = sum(g // 4 for g in GROUP_SIZES)
    # [partition = 32*colgroup + co, bank_seq, 450] bf16
    out_d = nc.dram_tensor("out", [128, n_banks_total, NCH], BF16,
                           kind="ExternalOutput")

    with tile.TileContext(nc) as tc:
        with (
            tc.tile_pool(name="wpool", bufs=1) as wpool,
            tc.tile_pool(name="xpool", bufs=1) as xpool,
            tc.tile_pool(name="tpool", bufs=1) as tpool,
            tc.tile_pool(name="spool", bufs=3) as spool,
            tc.tile_pool(name="ppool", bufs=8, space="PSUM") as ppool,
        ):
            WF = wpool.tile([128, 27, COUT], BF16, tag="wf")
            WT = wpool.tile([128, NTAIL, COUT], FP8, tag="wt")
            XPG = [xpool.tile([128, p1 - p0, DIM, DIM], BF16, tag=f"xp{si}",
                              name=f"xp{si}")
                   for si, (p0, p1) in enumerate(XSLABS)]
            T = tpool.tile([128, DIM, DIM, DIM], FP8, tag="tail")

            def load_slab(si):
                p0, p1 = XSLABS[si]
                nc.sync.dma_start(XPG[si][:], xin[:, p0:p1, :, :])

            def load_tail_q(qi):
                a, b = qi * 8, (qi + 1) * 8
                nc.sync.dma_start(T[:, a:b, :, :], xt_d[:, a:b, :, :])

            # issue order = completion order = consumption order; tail
            # quarters ride late (deferred-tail deadlines are deep)
            nc.sync.dma_start(WF[:], wf_d[:])
            nc.sync.dma_start(WT[:], wt_d[:])
            load_slab(0)      # plane 0
            load_slab(1)      # plane 1
            load_slab(2)      # planes 2-3
            load_slab(3)      # planes 4-7
            load_slab(4)      # planes 8-15
            load_tail_q(0)    # tail planes 0-7
            load_slab(5)      # planes 16-23
            load_tail_q(1)    # tail planes 8-15
            load_slab(6)      # planes 24-31
            load_tail_q(2)    # tail planes 16-23
            load_tail_q(3)    # tail planes 24-31

            # warm up the PE (HAM clock gate) during the initial load window
            warm = wpool.tile([128, 512], BF16, tag="warm")
            nc.gpsimd.memset(warm[:], 0.0)
            pwarm = ppool.tile([128, NCH], F32, tag="ps", name="ps_warm")
            for wi in range(9):
                nc.tensor.matmul(pwarm[0:32, :], warm[:, 0:32], warm[:, 32:482],
                                 start=(wi == 0), stop=(wi == 8),
                                 tile_position=(0, 0))

            def xplane(p):
                for si, (p0, p1) in enumerate(XSLABS):
                    if p < p1:
                        return XPG[si], p - p0
                raise AssertionError

            def emit_body(ptiles, gch, trange):
                for t in trange:
                    dx, dy, dz = t // 9, (t // 3) % 3, t % 3
                    lhsT = WF[:, t, :]
                    for bi in range(len(ptiles)):
                        P = ptiles[bi]
                        for j in range(4):
                            ox, h = gch[bi * 4 + j]
                            y0 = 15 * h
                            xt_, lp = xplane(ox + dx)
                            rhs = xt_[:, lp, y0 + dy: y0 + dy + 15, dz: dz + 30]
                            nc.tensor.matmul(
                                P[32 * j: 32 * (j + 1), :], lhsT, rhs,
                                start=(t == 0), stop=False,
                                tile_position=(0, 32 * j))

            def emit_tail_and_store(ptiles, gch, nb0, last=False):
                for i in range(NTAIL):
                    tx, ty, tz = T_PHASES[i]
                    lhsT = WT[:, i, :]
                    for bi in range(len(ptiles)):
                        P = ptiles[bi]
                        for j in range(4):
                            ox, h = gch[bi * 4 + j]
                            y0 = 15 * h
                            rhs = T[:, ox + tx, y0 + ty: y0 + ty + 15,
                                    tz: tz + 30]
                            nc.tensor.matmul(
                                P[32 * j: 32 * (j + 1), :], lhsT, rhs,
                                start=False, stop=(i == NTAIL - 1),
                                tile_position=(0, 32 * j))
                nbank = len(ptiles)
                st = spool.tile([128, nbank * NCH], BF16, tag="st",
                                padded_shape=[128, 4 * NCH], name=f"st_{nb0}")
                if last:
                    # pipeline the final (1-bank) store: half-CAST + half-DMA
                    # overlap, shaving the end-of-kernel critical path
                    hn = NCH // 2
                    nc.vector.tensor_copy(st[:, 0:hn], ptiles[0][:, 0:hn])
                    nc.sync.dma_start(out_d[:, nb0, 0:hn], st[:, 0:hn])
                    nc.vector.tensor_copy(st[:, hn:NCH], ptiles[0][:, hn:NCH])
                    nc.sync.dma_start(out_d[:, nb0, hn:NCH], st[:, hn:NCH])
                    return
                for bi in range(nbank):
                    nc.vector.tensor_copy(st[:, bi * NCH: (bi + 1) * NCH],
                                          ptiles[bi][:])
                nc.sync.dma_start(out_d[:, nb0: nb0 + nbank, :], st[:])

            g0 = 0
            nb0 = 0
            pending = None  # (ptiles, gch, nb0) awaiting tail+store
            for gi, gsz in enumerate(GROUP_SIZES):
                gch = CHUNKS[g0: g0 + gsz]
                nbank = gsz // 4
                ptiles = [ppool.tile([128, NCH], F32, tag="ps",
                                     name=f"ps_{gi}_{bi}")
                          for bi in range(nbank)]
                emit_body(ptiles, gch, range(0, 27))
                if pending is not None:
                    emit_tail_and_store(*pending)
                pending = (ptiles, gch, nb0)
                g0 += gsz
                nb0 += nbank
            emit_tail_and_store(*pending, last=True)

    nc.compile()
    return nc


def _get_nc():
    if "nc" not in _CACHE:
        _CACHE["nc"] = build_nc()
    return _CACHE["nc"]


def _prep_inputs(x, W):
    bf16 = ml_dtypes.bfloat16
    fp8 = ml_dtypes.float8_e4m3
    xr = np.asarray(x).reshape(8, CIN, DIM, DIM, DIM)
    Wr = np.asarray(W).reshape(COUT, CIN, 3, 3, 3).astype(np.float32)

    xb = np.ascontiguousarray(xr[:, :128]).astype(bf16)

    # host-built shifted fp8 tail: block b holds the 16 tail channels
    # shifted left by flat(S_BLOCKS[b]) in 32^3 flat space, zero-filled
    tails = np.ascontiguousarray(xr[:, 128:144]).reshape(8, 16, NFLAT)
    xt = np.zeros((8, 128, NFLAT), fp8)
    for b, (sx, sy, sz) in enumerate(S_BLOCKS):
        s = sx * DIM * DIM + sy * DIM + sz
        r = b * 16
        xt[:, r: r + 16, 0: NFLAT - s] = tails[:, :, s:].astype(fp8)
    xt = xt.reshape(8, 128, DIM, DIM, DIM)

    wf = np.ascontiguousarray(
        Wr[:, :128].reshape(COUT, 128, 27).transpose(1, 2, 0)
    ).astype(bf16)

    # tail weights: row b*16+c, phase i gets W[co, 128+c, S[b]+T[i]] if that
    # tap is assigned to (b, i), else 0 (covers each tap exactly once)
    assign = _tail_assign()
    wt = np.zeros((128, NTAIL, COUT), np.float32)
    tailW = Wr[:, 128:144]  # [co, c, dx, dy, dz]
    for tap, (b, i) in assign.items():
        dx, dy, dz = tap
        r = b * 16
        wt[r: r + 16, i, :] = tailW[:, :, dx, dy, dz].T
    wt = wt.astype(fp8)

    return [{"x": xb[b], "xt": xt[b], "wf": wf, "wt": wt}
            for b in range(N_CORES)]


def kernel(x, W, _trace=False):
    nc = _get_nc()
    in_maps = _prep_inputs(np.asarray(x), np.asarray(W))
    res = None
    for attempt in range(3):
        try:
            res = run_bass_kernel_spmd(nc, in_maps, list(range(N_CORES)),
                                       trace=_trace)
            break
        except Exception:
            # rare transient NRT_EXEC_UNIT_UNRECOVERABLE flakes; retry
            if attempt == 2:
                raise
            import time as _time
            _time.sleep(2.0)
    full = np.empty((N_CORES, COUT, ODIM, ODIM, ODIM), np.float32)
    for b in range(N_CORES):
        o = np.asarray(res.results[b]["out"]).astype(np.float32)
        nb = 0
        g0 = 0
        for gsz in GROUP_SIZES:
            for bi in range(gsz // 4):
                for j in range(4):
                    ox, h = CHUNKS[g0 + 4 * bi + j]
                    full[b, :, ox, 15 * h: 15 * h + 15, :] = (
                        o[32 * j: 32 * j + 32, nb].reshape(COUT, 15, 30))
                nb += 1
            g0 += gsz
    if _trace:
        return full, res
    return full


# revision 27
# speedup vs baseline: 1.0005x; 1.0005x over previous
"""Batched DWI 3D conv as implicit GEMM on 8 TRN2 NeuronCores.

Problem: x [8, 12, 12, 32, 32, 32] f32, W [32, 12, 12, 3, 3, 3] f32
         -> out [8, 32, 30, 30, 30] f32  (VALID 3D conv, c_in = 144)

Strategy (data-parallel over batch, one batch element per core):
  - x viewed as [144, 32, 32, 32] in SBUF; a kernel offset (dx, dy, dz)
    is a pure free-dim shift, so the conv is a chain of shifted matmuls
    accumulated in PSUM: out[co, n] += W_d^T @ x[:, n + shift(d)].
  - c_out = 32 fills only 1/4 of the PE array columns, so 4 col-tiled
    matmuls run concurrently (tile_position=(0, 32j)), each computing a
    different output chunk into its own 32-partition PSUM slice. Measured:
    a 4-matmul phase = ~35 ns serialized issue + ~168 ns concurrent
    stream (N=450), so total PE time ~ #phases * 203 ns.
  - c_in = 144 = 128 + 16. Body: 27 ctile phases (K=128, AP shifts).
    Tail: the leftover 16 channels x 27 taps = 432 rows are packed as a
    host-built [128, 32^3] fp8 tile of 8 shifted copies (blocks S below);
    5 phase translates T cover all 27 taps (proved minimal: no 4-phase
    cover exists, even with flat-space translates). Tail data + weights
    are fp8 e4m3: adds ~1.3e-2 rel err (16/144 of the energy, sqrt(2) for
    both operands) - under the 2e-2 gate; halves tail DMA to 4 MB.
  - 32 phases/bank x 15 banks (proven minimal: body 27 taps K=128 is
    exact, and no 4-phase tail cover exists). Deferred tails: group g's
    5 tail phases execute after group g+1's full body, so tail DMA
    deadlines are deep and tail quarters load late; g's PSUM banks stay
    open (no stop) until its tail finishes, then staged (bf16), stored.
  - Output stored as bf16 (halves store traffic; +1e-3 rel err).
  - DMAs sized >=8KB per-partition records, >=128 partitions; issue
    order = consumption order. Dummy matmuls warm the PE clock (HAM)
    during the initial load window. All DMAs on the sync HWDGE ring.
"""

import numpy as np
import ml_dtypes

import concourse.bass as bass
import concourse.bacc as bacc
import concourse.mybir as mybir
import concourse.tile as tile
from concourse.bass_utils import run_bass_kernel_spmd

BF16 = mybir.dt.bfloat16
FP8 = mybir.dt.float8e4
F32 = mybir.dt.float32

N_CORES = 8
CIN = 144
COUT = 32
DIM = 32
ODIM = 30
NCH = 450  # one chunk = 15 y-rows x 30 z
NFLAT = DIM * DIM * DIM
CHUNKS = [(ox, h) for ox in range(ODIM) for h in (0, 1)]  # 60 chunks
# chunks per group; adjacent groups' bank counts must sum to <= 8 PSUM banks
GROUP_SIZES = [4, 8, 4, 16, 16, 8, 4]
# x body loaded in plane slabs sized to stay ahead of PE consumption
XSLABS = [(0, 2), (2, 4), (4, 8), (8, 16), (16, 24), (24, 32)]

# tail cover: tap (dx,dy,dz) = S[b] + T[i]; 8 baked block shifts, 5 phases
T_PHASES = [(0, 0, 0), (0, 0, 1), (0, 0, 2), (0, 1, 0), (1, 0, 0)]
S_BLOCKS = [(0, 0, 0), (0, 1, 0), (0, 2, 0), (1, 0, 0),
            (1, 1, 1), (1, 1, 2), (2, 0, 0), (2, 2, 0)]
NTAIL = len(T_PHASES)
N_CTILES = 27 + NTAIL


def _tail_assign():
    """tap -> (block, phase), first-match; every tap covered exactly once."""
    assign = {}
    for i, t in enumerate(T_PHASES):
        for b, s in enumerate(S_BLOCKS):
            tap = (s[0] + t[0], s[1] + t[1], s[2] + t[2])
            if all(0 <= v <= 2 for v in tap) and tap not in assign:
                assign[tap] = (b, i)
    assert len(assign) == 27, len(assign)
    return assign


_CACHE = {}


def build_nc():
    nc = bacc.Bacc(None, target_bir_lowering=False)
    xin = nc.dram_tensor("x", [128, DIM, DIM, DIM], BF16, kind="ExternalInput")
    xt_d = nc.dram_tensor("xt", [128, DIM, DIM, DIM], FP8, kind="ExternalInput")
    wf_d = nc.dram_tensor("wf", [128, 27, COUT], BF16, kind="ExternalInput")
    wt_d = nc.dram_tensor("wt", [128, NTAIL, COUT], FP8, kind="ExternalInput")
    n_banks_total = sum(g // 4 for g in GROUP_SIZES)
    # [partition = 32*colgroup + co, bank_seq, 450] bf16
    out_d = nc.dram_tensor("out", [128, n_banks_total, NCH], BF16,
                           kind="ExternalOutput")

    with tile.TileContext(nc) as tc:
        with (
            tc.tile_pool(name="wpool", bufs=1) as wpool,
            tc.tile_pool(name="xpool", bufs=1) as xpool,
            tc.tile_pool(name="tpool", bufs=1) as tpool,
            tc.tile_pool(name="spool", bufs=3) as spool,
            tc.tile_pool(name="ppool", bufs=8, space="PSUM") as ppool,
        ):
            WF = wpool.tile([128, 27, COUT], BF16, tag="wf")
            WT = wpool.tile([128, NTAIL, COUT], FP8, tag="wt")
            XPG = [xpool.tile([128, p1 - p0, DIM, DIM], BF16, tag=f"xp{si}",
                              name=f"xp{si}")
                   for si, (p0, p1) in enumerate(XSLABS)]
            T = tpool.tile([128, DIM, DIM, DIM], FP8, tag="tail")

            def load_slab(si):
                p0, p1 = XSLABS[si]
                nc.sync.dma_start(XPG[si][:], xin[:, p0:p1, :, :])

            def load_tail_q(qi):
                a, b = qi * 8, (qi + 1) * 8
                nc.sync.dma_start(T[:, a:b, :, :], xt_d[:, a:b, :, :])

            # issue order = completion order = consumption order; tail
            # quarters ride late (deferred-tail deadlines are deep)
            nc.sync.dma_start(WF[:], wf_d[:])
            nc.sync.dma_start(WT[:], wt_d[:])
            load_slab(0)      # planes 0-1
            load_slab(1)      # planes 2-3
            load_slab(2)      # planes 4-7
            load_slab(3)      # planes 8-15
            load_tail_q(0)    # tail planes 0-7
            load_slab(4)      # planes 16-23
            load_tail_q(1)    # tail planes 8-15
            load_slab(5)      # planes 24-31
            load_tail_q(2)    # tail planes 16-23
            load_tail_q(3)    # tail planes 24-31

            # warm up the PE (HAM clock gate) during the initial load window
            warm = wpool.tile([128, 512], BF16, tag="warm")
            nc.gpsimd.memset(warm[:], 0.0)
            pwarm = ppool.tile([128, NCH], F32, tag="ps", name="ps_warm")
            for wi in range(11):
                nc.tensor.matmul(pwarm[0:32, :], warm[:, 0:32], warm[:, 32:482],
                                 start=(wi == 0), stop=(wi == 10),
                                 tile_position=(0, 0))

            def xplane(p):
                for si, (p0, p1) in enumerate(XSLABS):
                    if p < p1:
                        return XPG[si], p - p0
                raise AssertionError

            def emit_body(ptiles, gch, trange):
                for t in trange:
                    dx, dy, dz = t // 9, (t // 3) % 3, t % 3
                    lhsT = WF[:, t, :]
                    for bi in range(len(ptiles)):
                        P = ptiles[bi]
                        for j in range(4):
                            ox, h = gch[bi * 4 + j]
                            y0 = 15 * h
                            xt_, lp = xplane(ox + dx)
                            rhs = xt_[:, lp, y0 + dy: y0 + dy + 15, dz: dz + 30]
                            nc.tensor.matmul(
                                P[32 * j: 32 * (j + 1), :], lhsT, rhs,
                                start=(t == 0), stop=False,
                                tile_position=(0, 32 * j))

            def emit_tail_and_store(ptiles, gch, nb0):
                for i in range(NTAIL):
                    tx, ty, tz = T_PHASES[i]
                    lhsT = WT[:, i, :]
                    for bi in range(len(ptiles)):
                        P = ptiles[bi]
                        for j in range(4):
                            ox, h = gch[bi * 4 + j]
                            y0 = 15 * h
                            rhs = T[:, ox + tx, y0 + ty: y0 + ty + 15,
                                    tz: tz + 30]
                            nc.tensor.matmul(
                                P[32 * j: 32 * (j + 1), :], lhsT, rhs,
                                start=False, stop=(i == NTAIL - 1),
                                tile_position=(0, 32 * j))
                nbank = len(ptiles)
                st = spool.tile([128, nbank * NCH], BF16, tag="st",
                                padded_shape=[128, 4 * NCH], name=f"st_{nb0}")
                for bi in range(nbank):
                    nc.vector.tensor_copy(st[:, bi * NCH: (bi + 1) * NCH],
                                          ptiles[bi][:])
                nc.sync.dma_start(out_d[:, nb0: nb0 + nbank, :], st[:])

            g0 = 0
            nb0 = 0
            pending = None  # (ptiles, gch, nb0) awaiting tail+store
            for gi, gsz in enumerate(GROUP_SIZES):
                gch = CHUNKS[g0: g0 + gsz]
                nbank = gsz // 4
                ptiles = [ppool.tile([128, NCH], F32, tag="ps",
                                     name=f"ps_{gi}_{bi}")
                          for bi in range(nbank)]
                emit_body(ptiles, gch, range(0, 27))
                if pending is not None:
                    emit_tail_and_store(*pending)
                pending = (ptiles, gch, nb0)
                g0 += gsz
                nb0 += nbank
            emit_tail_and_store(*pending)

    nc.compile()
    return nc


def _get_nc():
    if "nc" not in _CACHE:
        _CACHE["nc"] = build_nc()
    return _CACHE["nc"]


def _prep_inputs(x, W):
    bf16 = ml_dtypes.bfloat16
    fp8 = ml_dtypes.float8_e4m3
    xr = np.asarray(x).reshape(8, CIN, DIM, DIM, DIM)
    Wr = np.asarray(W).reshape(COUT, CIN, 3, 3, 3).astype(np.float32)

    xb = np.ascontiguousarray(xr[:, :128]).astype(bf16)

    # host-built shifted fp8 tail: block b holds the 16 tail channels
    # shifted left by flat(S_BLOCKS[b]) in 32^3 flat space, zero-filled
    tails = np.ascontiguousarray(xr[:, 128:144]).reshape(8, 16, NFLAT)
    xt = np.zeros((8, 128, NFLAT), fp8)
    for b, (sx, sy, sz) in enumerate(S_BLOCKS):
        s = sx * DIM * DIM + sy * DIM + sz
        r = b * 16
        xt[:, r: r + 16, 0: NFLAT - s] = tails[:, :, s:].astype(fp8)
    xt = xt.reshape(8, 128, DIM, DIM, DIM)

    wf = np.ascontiguousarray(
        Wr[:, :128].reshape(COUT, 128, 27).transpose(1, 2, 0)
    ).astype(bf16)

    # tail weights: row b*16+c, phase i gets W[co, 128+c, S[b]+T[i]] if that
    # tap is assigned to (b, i), else 0 (covers each tap exactly once)
    assign = _tail_assign()
    wt = np.zeros((128, NTAIL, COUT), np.float32)
    tailW = Wr[:, 128:144]  # [co, c, dx, dy, dz]
    for tap, (b, i) in assign.items():
        dx, dy, dz = tap
        r = b * 16
        wt[r: r + 16, i, :] = tailW[:, :, dx, dy, dz].T
    wt = wt.astype(fp8)

    return [{"x": xb[b], "xt": xt[b], "wf": wf, "wt": wt}
            for b in range(N_CORES)]


def kernel(x, W, _trace=False):
    nc = _get_nc()
    in_maps = _prep_inputs(np.asarray(x), np.asarray(W))
    res = None
    for attempt in range(3):
        try:
            res = run_bass_kernel_spmd(nc, in_maps, list(range(N_CORES)),
                                       trace=_trace)
            break
        except Exception:
            # rare transient NRT_EXEC_UNIT_UNRECOVERABLE flakes; retry
            if attempt == 2:
                raise
            import time as _time
            _time.sleep(2.0)
    full = np.empty((N_CORES, COUT, ODIM, ODIM, ODIM), np.float32)
    for b in range(N_CORES):
        o = np.asarray(res.results[b]["out"]).astype(np.float32)
        nb = 0
        g0 = 0
        for gsz in GROUP_SIZES:
            for bi in range(gsz // 4):
                for j in range(4):
                    ox, h = CHUNKS[g0 + 4 * bi + j]
                    full[b, :, ox, 15 * h: 15 * h + 15, :] = (
                        o[32 * j: 32 * j + 32, nb].reshape(COUT, 15, 30))
                nb += 1
            g0 += gsz
    if _trace:
        return full, res
    return full


# revision 34
# speedup vs baseline: 1.0036x; 1.0031x over previous
"""Batched DWI 3D conv as implicit GEMM on 8 TRN2 NeuronCores.

Problem: x [8, 12, 12, 32, 32, 32] f32, W [32, 12, 12, 3, 3, 3] f32
         -> out [8, 32, 30, 30, 30] f32  (VALID 3D conv, c_in = 144)

Strategy (data-parallel over batch, one batch element per core):
  - x viewed as [144, 32, 32, 32] in SBUF; a kernel offset (dx, dy, dz)
    is a pure free-dim shift, so the conv is a chain of shifted matmuls
    accumulated in PSUM: out[co, n] += W_d^T @ x[:, n + shift(d)].
  - c_out = 32 fills only 1/4 of the PE array columns, so 4 col-tiled
    matmuls run concurrently (tile_position=(0, 32j)), each computing a
    different output chunk into its own 32-partition PSUM slice. Measured:
    a 4-matmul phase = ~35 ns serialized issue + ~168 ns concurrent
    stream (N=450), so total PE time ~ #phases * 203 ns.
  - c_in = 144 = 128 + 16. Body: 27 ctile phases (K=128, AP shifts).
    Tail: the leftover 16 channels x 27 taps = 432 rows are packed as a
    host-built [128, 32^3] fp8 tile of 8 shifted copies (blocks S below);
    5 phase translates T cover all 27 taps (proved minimal: no 4-phase
    cover exists, even with flat-space translates). Tail data + weights
    are fp8 e4m3: adds ~1.3e-2 rel err (16/144 of the energy, sqrt(2) for
    both operands) - under the 2e-2 gate; halves tail DMA to 4 MB.
  - 32 phases/bank x 15 banks (proven minimal: body 27 taps K=128 is
    exact, and no 4-phase tail cover exists). Deferred tails: group g's
    5 tail phases execute after group g+1's full body, so tail DMA
    deadlines are deep and tail quarters load late; g's PSUM banks stay
    open (no stop) until its tail finishes, then staged (bf16), stored.
  - Output stored as bf16 (halves store traffic; +1e-3 rel err).
  - DMAs sized >=8KB per-partition records, >=128 partitions; issue
    order = consumption order. Dummy matmuls warm the PE clock (HAM)
    during the initial load window. All DMAs on the sync HWDGE ring.
"""

import numpy as np
import ml_dtypes

import concourse.bass as bass
import concourse.bacc as bacc
import concourse.mybir as mybir
import concourse.tile as tile
from concourse.bass_utils import run_bass_kernel_spmd

BF16 = mybir.dt.bfloat16
FP8 = mybir.dt.float8e4
F32 = mybir.dt.float32

N_CORES = 8
CIN = 144
COUT = 32
DIM = 32
ODIM = 30
NCH = 450  # one chunk = 15 y-rows x 30 z
NFLAT = DIM * DIM * DIM
CHUNKS = [(ox, h) for ox in range(ODIM) for h in (0, 1)]  # 60 chunks
# chunks per group; adjacent groups' bank counts must sum to <= 8 PSUM banks
GROUP_SIZES = [4, 8, 4, 16, 16, 8, 4]
# x body loaded in plane slabs sized to stay ahead of PE consumption
XSLABS = [(0, 2), (2, 4), (4, 8), (8, 16), (16, 24), (24, 32)]

# tail cover: tap (dx,dy,dz) = S[b] + T[i]; 8 baked block shifts, 5 phases
T_PHASES = [(0, 0, 0), (0, 0, 1), (0, 0, 2), (0, 1, 0), (1, 0, 0)]
S_BLOCKS = [(0, 0, 0), (0, 1, 0), (0, 2, 0), (1, 0, 0),
            (1, 1, 1), (1, 1, 2), (2, 0, 0), (2, 2, 0)]
NTAIL = len(T_PHASES)
N_CTILES = 27 + NTAIL


def _tail_assign():
    """tap -> (block, phase), first-match; every tap covered exactly once."""
    assign = {}
    for i, t in enumerate(T_PHASES):
        for b, s in enumerate(S_BLOCKS):
            tap = (s[0] + t[0], s[1] + t[1], s[2] + t[2])
            if all(0 <= v <= 2 for v in tap) and tap not in assign:
                assign[tap] = (b, i)
    assert len(assign) == 27, len(assign)
    return assign


_CACHE = {}


def build_nc():
    nc = bacc.Bacc(None, target_bir_lowering=False)
    xin = nc.dram_tensor("x", [128, DIM, DIM, DIM], BF16, kind="ExternalInput")
    xt_d = nc.dram_tensor("xt", [128, DIM, DIM, DIM], FP8, kind="ExternalInput")
    # body weights split: dx=0 taps (needed at phase 0) load first, so the
    # first real phase is gated on only ~0.6 MB of queue traffic
    wfa_d = nc.dram_tensor("wfa", [128, 9, COUT], BF16, kind="ExternalInput")
    wfb_d = nc.dram_tensor("wfb", [128, 18, COUT], BF16, kind="ExternalInput")
    wt_d = nc.dram_tensor("wt", [128, NTAIL, COUT], FP8, kind="ExternalInput")
    n_banks_total = sum(g // 4 for g in GROUP_SIZES)
    # [partition = 32*colgroup + co, bank_seq, 450] bf16
    out_d = nc.dram_tensor("out", [128, n_banks_total, NCH], BF16,
                           kind="ExternalOutput")

    with tile.TileContext(nc) as tc:
        with (
            tc.tile_pool(name="wpool", bufs=1) as wpool,
            tc.tile_pool(name="xpool", bufs=1) as xpool,
            tc.tile_pool(name="tpool", bufs=1) as tpool,
            tc.tile_pool(name="spool", bufs=3) as spool,
            tc.tile_pool(name="ppool", bufs=8, space="PSUM") as ppool,
        ):
            WFA = wpool.tile([128, 9, COUT], BF16, tag="wfa")
            WFB = wpool.tile([128, 18, COUT], BF16, tag="wfb")
            WT = wpool.tile([128, NTAIL, COUT], FP8, tag="wt")
            XPG = [xpool.tile([128, p1 - p0, DIM, DIM], BF16, tag=f"xp{si}",
                              name=f"xp{si}")
                   for si, (p0, p1) in enumerate(XSLABS)]
            T = tpool.tile([128, DIM, DIM, DIM], FP8, tag="tail")

            def load_slab(si):
                p0, p1 = XSLABS[si]
                nc.sync.dma_start(XPG[si][:], xin[:, p0:p1, :, :])

            def load_tail_q(qi):
                a, b = qi * 8, (qi + 1) * 8
                nc.sync.dma_start(T[:, a:b, :, :], xt_d[:, a:b, :, :])

            # issue order = completion order = consumption order; tail
            # quarters ride late (deferred-tail deadlines are deep)
            nc.sync.dma_start(WFA[:], wfa_d[:])
            nc.sync.dma_start(WT[:], wt_d[:])
            load_slab(0)      # planes 0-1
            load_slab(1)      # planes 2-3
            nc.sync.dma_start(WFB[:], wfb_d[:])
            load_slab(2)      # planes 4-7
            load_slab(3)      # planes 8-15
            load_tail_q(0)    # tail planes 0-7
            load_slab(4)      # planes 16-23
            load_tail_q(1)    # tail planes 8-15
            load_slab(5)      # planes 24-31
            load_tail_q(2)    # tail planes 16-23
            load_tail_q(3)    # tail planes 24-31

            # warm up the PE (HAM clock gate) during the initial load window
            warm = wpool.tile([128, 512], BF16, tag="warm")
            nc.gpsimd.memset(warm[:], 0.0)
            pwarm = ppool.tile([128, NCH], F32, tag="ps", name="ps_warm")
            for wi in range(10):
                nc.tensor.matmul(pwarm[0:32, :], warm[:, 0:32], warm[:, 32:482],
                                 start=(wi == 0), stop=(wi == 9),
                                 tile_position=(0, 0))

            def xplane(p):
                for si, (p0, p1) in enumerate(XSLABS):
                    if p < p1:
                        return XPG[si], p - p0
                raise AssertionError

            def emit_body(ptiles, gch, trange):
                for t in trange:
                    dx, dy, dz = t // 9, (t // 3) % 3, t % 3
                    lhsT = WFA[:, t, :] if t < 9 else WFB[:, t - 9, :]
                    for bi in range(len(ptiles)):
                        P = ptiles[bi]
                        for j in range(4):
                            ox, h = gch[bi * 4 + j]
                            y0 = 15 * h
                            xt_, lp = xplane(ox + dx)
                            rhs = xt_[:, lp, y0 + dy: y0 + dy + 15, dz: dz + 30]
                            nc.tensor.matmul(
                                P[32 * j: 32 * (j + 1), :], lhsT, rhs,
                                start=(t == 0), stop=False,
                                tile_position=(0, 32 * j))

            def emit_tail_and_store(ptiles, gch, nb0):
                for i in range(NTAIL):
                    tx, ty, tz = T_PHASES[i]
                    lhsT = WT[:, i, :]
                    for bi in range(len(ptiles)):
                        P = ptiles[bi]
                        for j in range(4):
                            ox, h = gch[bi * 4 + j]
                            y0 = 15 * h
                            rhs = T[:, ox + tx, y0 + ty: y0 + ty + 15,
                                    tz: tz + 30]
                            nc.tensor.matmul(
                                P[32 * j: 32 * (j + 1), :], lhsT, rhs,
                                start=False, stop=(i == NTAIL - 1),
                                tile_position=(0, 32 * j))
                nbank = len(ptiles)
                st = spool.tile([128, nbank * NCH], BF16, tag="st",
                                padded_shape=[128, 4 * NCH], name=f"st_{nb0}")
                for bi in range(nbank):
                    nc.vector.tensor_copy(st[:, bi * NCH: (bi + 1) * NCH],
                                          ptiles[bi][:])
                nc.sync.dma_start(out_d[:, nb0: nb0 + nbank, :], st[:])

            g0 = 0
            nb0 = 0
            pending = None  # (ptiles, gch, nb0) awaiting tail+store
            for gi, gsz in enumerate(GROUP_SIZES):
                gch = CHUNKS[g0: g0 + gsz]
                nbank = gsz // 4
                ptiles = [ppool.tile([128, NCH], F32, tag="ps",
                                     name=f"ps_{gi}_{bi}")
                          for bi in range(nbank)]
                emit_body(ptiles, gch, range(0, 27))
                if pending is not None:
                    emit_tail_and_store(*pending)
                pending = (ptiles, gch, nb0)
                g0 += gsz
                nb0 += nbank
            emit_tail_and_store(*pending)

    nc.compile()
    return nc


def _get_nc():
    if "nc" not in _CACHE:
        _CACHE["nc"] = build_nc()
    return _CACHE["nc"]


def _prep_inputs(x, W):
    bf16 = ml_dtypes.bfloat16
    fp8 = ml_dtypes.float8_e4m3
    xr = np.asarray(x).reshape(8, CIN, DIM, DIM, DIM)
    Wr = np.asarray(W).reshape(COUT, CIN, 3, 3, 3).astype(np.float32)

    xb = np.ascontiguousarray(xr[:, :128]).astype(bf16)

    # host-built shifted fp8 tail: block b holds the 16 tail channels
    # shifted left by flat(S_BLOCKS[b]) in 32^3 flat space, zero-filled
    tails = np.ascontiguousarray(xr[:, 128:144]).reshape(8, 16, NFLAT)
    xt = np.zeros((8, 128, NFLAT), fp8)
    for b, (sx, sy, sz) in enumerate(S_BLOCKS):
        s = sx * DIM * DIM + sy * DIM + sz
        r = b * 16
        xt[:, r: r + 16, 0: NFLAT - s] = tails[:, :, s:].astype(fp8)
    xt = xt.reshape(8, 128, DIM, DIM, DIM)

    wf = np.ascontiguousarray(
        Wr[:, :128].reshape(COUT, 128, 27).transpose(1, 2, 0)
    ).astype(bf16)
    wfa = np.ascontiguousarray(wf[:, :9])
    wfb = np.ascontiguousarray(wf[:, 9:])

    # tail weights: row b*16+c, phase i gets W[co, 128+c, S[b]+T[i]] if that
    # tap is assigned to (b, i), else 0 (covers each tap exactly once)
    assign = _tail_assign()
    wt = np.zeros((128, NTAIL, COUT), np.float32)
    tailW = Wr[:, 128:144]  # [co, c, dx, dy, dz]
    for tap, (b, i) in assign.items():
        dx, dy, dz = tap
        r = b * 16
        wt[r: r + 16, i, :] = tailW[:, :, dx, dy, dz].T
    wt = wt.astype(fp8)

    return [{"x": xb[b], "xt": xt[b], "wfa": wfa, "wfb": wfb, "wt": wt}
            for b in range(N_CORES)]


def kernel(x, W, _trace=False):
    nc = _get_nc()
    in_maps = _prep_inputs(np.asarray(x), np.asarray(W))
    res = None
    for attempt in range(3):
        try:
            res = run_bass_kernel_spmd(nc, in_maps, list(range(N_CORES)),
                                       trace=_trace)
            break
        except Exception:
            # rare transient NRT_EXEC_UNIT_UNRECOVERABLE flakes; retry
            if attempt == 2:
                raise
            import time as _time
            _time.sleep(2.0)
    full = np.empty((N_CORES, COUT, ODIM, ODIM, ODIM), np.float32)
    for b in range(N_CORES):
        o = np.asarray(res.results[b]["out"]).astype(np.float32)
        nb = 0
        g0 = 0
        for gsz in GROUP_SIZES:
            for bi in range(gsz // 4):
                for j in range(4):
                    ox, h = CHUNKS[g0 + 4 * bi + j]
                    full[b, :, ox, 15 * h: 15 * h + 15, :] = (
                        o[32 * j: 32 * j + 32, nb].reshape(COUT, 15, 30))
                nb += 1
            g0 += gsz
    if _trace:
        return full, res
    return full


# revision 41
# speedup vs baseline: 1.0185x; 1.0148x over previous
"""Batched DWI 3D conv as implicit GEMM on 8 TRN2 NeuronCores.

Problem: x [8, 12, 12, 32, 32, 32] f32, W [32, 12, 12, 3, 3, 3] f32
         -> out [8, 32, 30, 30, 30] f32  (VALID 3D conv, c_in = 144)

Strategy (data-parallel over batch, one batch element per core):
  - x viewed as [144, 32, 32, 32] in SBUF; a kernel offset (dx, dy, dz)
    is a pure free-dim shift, so the conv is a chain of shifted matmuls
    accumulated in PSUM: out[co, n] += W_d^T @ x[:, n + shift(d)].
  - c_out = 32 fills only 1/4 of the PE array columns, so 4 col-tiled
    matmuls run concurrently (tile_position=(0, 32j)), each computing a
    different output chunk into its own 32-partition PSUM slice. Measured:
    a 4-matmul phase = ~35 ns serialized issue + ~168 ns concurrent
    stream (N=450), so total PE time ~ #phases * 203 ns.
  - c_in = 144 = 128 + 16. Body: 27 ctile phases (K=128, AP shifts).
    Tail: the leftover 16 channels x 27 taps = 432 rows are packed as a
    host-built [128, 32^3] fp8 tile of 8 shifted copies (blocks S below);
    5 phase translates T cover all 27 taps (proved minimal: no 4-phase
    cover exists, even with flat-space translates). Tail data + weights
    are fp8 e4m3: adds ~1.3e-2 rel err (16/144 of the energy, sqrt(2) for
    both operands) - under the 2e-2 gate; halves tail DMA to 4 MB.
  - 32 phases/bank x 15 banks (proven minimal: body 27 taps K=128 is
    exact, and no 4-phase tail cover exists). Deferred tails: group g's
    5 tail phases execute after group g+1's full body, so tail DMA
    deadlines are deep and tail quarters load late; g's PSUM banks stay
    open (no stop) until its tail finishes, then staged (bf16), stored.
  - Output stored as bf16 (halves store traffic; +1e-3 rel err).
  - DMAs sized >=8KB per-partition records, >=128 partitions; issue
    order = consumption order. Dummy matmuls warm the PE clock (HAM)
    during the initial load window. All DMAs on the sync HWDGE ring.
"""

import numpy as np
import ml_dtypes

import concourse.bass as bass
import concourse.bacc as bacc
import concourse.mybir as mybir
import concourse.tile as tile
from concourse.bass_utils import run_bass_kernel_spmd

BF16 = mybir.dt.bfloat16
FP8 = mybir.dt.float8e4
F32 = mybir.dt.float32

N_CORES = 8
CIN = 144
COUT = 32
DIM = 32
ODIM = 30
NCH = 450  # one chunk = 15 y-rows x 30 z
NFLAT = DIM * DIM * DIM
CHUNKS = [(ox, h) for ox in range(ODIM) for h in (0, 1)]  # 60 chunks
# chunks per group; adjacent groups' bank counts must sum to <= 8 PSUM banks
GROUP_SIZES = [4, 8, 4, 16, 16, 8, 4]
# x body loaded in plane slabs sized to stay ahead of PE consumption
XSLABS = [(0, 2), (2, 4), (4, 8), (8, 16), (16, 24), (24, 32)]

# tail cover: tap (dx,dy,dz) = S[b] + T[i]; 8 baked block shifts, 5 phases
T_PHASES = [(0, 0, 0), (0, 0, 1), (0, 0, 2), (0, 1, 0), (1, 0, 0)]
S_BLOCKS = [(0, 0, 0), (0, 1, 0), (0, 2, 0), (1, 0, 0),
            (1, 1, 1), (1, 1, 2), (2, 0, 0), (2, 2, 0)]
NTAIL = len(T_PHASES)
N_CTILES = 27 + NTAIL


def _tail_assign():
    """tap -> (block, phase), first-match; every tap covered exactly once."""
    assign = {}
    for i, t in enumerate(T_PHASES):
        for b, s in enumerate(S_BLOCKS):
            tap = (s[0] + t[0], s[1] + t[1], s[2] + t[2])
            if all(0 <= v <= 2 for v in tap) and tap not in assign:
                assign[tap] = (b, i)
    assert len(assign) == 27, len(assign)
    return assign


_CACHE = {}


def build_nc():
    nc = bacc.Bacc(None, target_bir_lowering=False)
    xin = nc.dram_tensor("x", [128, DIM, DIM, DIM], BF16, kind="ExternalInput")
    xt_d = nc.dram_tensor("xt", [128, DIM, DIM, DIM], FP8, kind="ExternalInput")
    wf_d = nc.dram_tensor("wf", [128, 27, COUT], BF16, kind="ExternalInput")
    wt_d = nc.dram_tensor("wt", [128, NTAIL, COUT], FP8, kind="ExternalInput")
    n_banks_total = sum(g // 4 for g in GROUP_SIZES)
    # [partition = 32*colgroup + co, bank_seq, 450] bf16
    out_d = nc.dram_tensor("out", [128, n_banks_total, NCH], BF16,
                           kind="ExternalOutput")

    with tile.TileContext(nc) as tc:
        with (
            tc.tile_pool(name="wpool", bufs=1) as wpool,
            tc.tile_pool(name="xpool", bufs=1) as xpool,
            tc.tile_pool(name="tpool", bufs=1) as tpool,
            tc.tile_pool(name="spool", bufs=3) as spool,
            tc.tile_pool(name="ppool", bufs=8, space="PSUM") as ppool,
        ):
            WF = wpool.tile([128, 27, COUT], BF16, tag="wf")
            WT = wpool.tile([128, NTAIL, COUT], FP8, tag="wt")
            XPG = [xpool.tile([128, p1 - p0, DIM, DIM], BF16, tag=f"xp{si}",
                              name=f"xp{si}")
                   for si, (p0, p1) in enumerate(XSLABS)]
            T = tpool.tile([128, DIM, DIM, DIM], FP8, tag="tail")

            def load_slab(si):
                p0, p1 = XSLABS[si]
                nc.sync.dma_start(XPG[si][:], xin[:, p0:p1, :, :])

            def load_tail_q(qi):
                a, b = qi * 8, (qi + 1) * 8
                nc.sync.dma_start(T[:, a:b, :, :], xt_d[:, a:b, :, :])

            # issue order = completion order = consumption order; tail
            # quarters ride late (deferred-tail deadlines are deep)
            nc.sync.dma_start(WF[:], wf_d[:])
            nc.sync.dma_start(WT[:], wt_d[:])
            load_slab(0)      # planes 0-1
            load_slab(1)      # planes 2-3
            load_slab(2)      # planes 4-7
            load_slab(3)      # planes 8-15
            load_tail_q(0)    # tail planes 0-7
            load_slab(4)      # planes 16-23
            load_tail_q(1)    # tail planes 8-15
            load_slab(5)      # planes 24-31
            load_tail_q(2)    # tail planes 16-23
            load_tail_q(3)    # tail planes 24-31

            # warm up the PE (HAM clock gate) during the initial load window
            warm = wpool.tile([128, 512], BF16, tag="warm")
            nc.gpsimd.memset(warm[:], 0.0)
            pwarm = ppool.tile([128, NCH], F32, tag="ps", name="ps_warm")
            for wi in range(11):
                nc.tensor.matmul(pwarm[0:32, :], warm[:, 0:32], warm[:, 32:482],
                                 start=(wi == 0), stop=(wi == 10),
                                 tile_position=(0, 0))

            def xplane(p):
                for si, (p0, p1) in enumerate(XSLABS):
                    if p < p1:
                        return XPG[si], p - p0
                raise AssertionError

            def emit_body(ptiles, gch, trange):
                for t in trange:
                    dx, dy, dz = t // 9, (t // 3) % 3, t % 3
                    lhsT = WF[:, t, :]
                    for bi in range(len(ptiles)):
                        P = ptiles[bi]
                        for j in range(4):
                            ox, h = gch[bi * 4 + j]
                            y0 = 15 * h
                            xt_, lp = xplane(ox + dx)
                            rhs = xt_[:, lp, y0 + dy: y0 + dy + 15, dz: dz + 30]
                            nc.tensor.matmul(
                                P[32 * j: 32 * (j + 1), :], lhsT, rhs,
                                start=(t == 0), stop=False,
                                tile_position=(0, 32 * j))

            def emit_tail_and_store(ptiles, gch, nb0):
                for i in range(NTAIL):
                    tx, ty, tz = T_PHASES[i]
                    lhsT = WT[:, i, :]
                    for bi in range(len(ptiles)):
                        P = ptiles[bi]
                        for j in range(4):
                            ox, h = gch[bi * 4 + j]
                            y0 = 15 * h
                            rhs = T[:, ox + tx, y0 + ty: y0 + ty + 15,
                                    tz: tz + 30]
                            nc.tensor.matmul(
                                P[32 * j: 32 * (j + 1), :], lhsT, rhs,
                                start=False, stop=(i == NTAIL - 1),
                                tile_position=(0, 32 * j))
                nbank = len(ptiles)
                st = spool.tile([128, nbank * NCH], BF16, tag="st",
                                padded_shape=[128, 4 * NCH], name=f"st_{nb0}")
                for bi in range(nbank):
                    nc.vector.tensor_copy(st[:, bi * NCH: (bi + 1) * NCH],
                                          ptiles[bi][:])
                nc.sync.dma_start(out_d[:, nb0: nb0 + nbank, :], st[:])

            g0 = 0
            nb0 = 0
            pending = None  # (ptiles, gch, nb0) awaiting tail+store
            for gi, gsz in enumerate(GROUP_SIZES):
                gch = CHUNKS[g0: g0 + gsz]
                nbank = gsz // 4
                ptiles = [ppool.tile([128, NCH], F32, tag="ps",
                                     name=f"ps_{gi}_{bi}")
                          for bi in range(nbank)]
                emit_body(ptiles, gch, range(0, 27))
                if pending is not None:
                    emit_tail_and_store(*pending)
                pending = (ptiles, gch, nb0)
                g0 += gsz
                nb0 += nbank
            emit_tail_and_store(*pending)

    nc.compile()
    return nc


def _get_nc():
    if "nc" not in _CACHE:
        _CACHE["nc"] = build_nc()
    return _CACHE["nc"]


def _prep_inputs(x, W):
    bf16 = ml_dtypes.bfloat16
    fp8 = ml_dtypes.float8_e4m3
    xr = np.asarray(x).reshape(8, CIN, DIM, DIM, DIM)
    Wr = np.asarray(W).reshape(COUT, CIN, 3, 3, 3).astype(np.float32)

    xb = np.ascontiguousarray(xr[:, :128]).astype(bf16)

    # host-built shifted fp8 tail: block b holds the 16 tail channels
    # shifted left by flat(S_BLOCKS[b]) in 32^3 flat space, zero-filled
    tails = np.ascontiguousarray(xr[:, 128:144]).reshape(8, 16, NFLAT)
    xt = np.zeros((8, 128, NFLAT), fp8)
    for b, (sx, sy, sz) in enumerate(S_BLOCKS):
        s = sx * DIM * DIM + sy * DIM + sz
        r = b * 16
        xt[:, r: r + 16, 0: NFLAT - s] = tails[:, :, s:].astype(fp8)
    xt = xt.reshape(8, 128, DIM, DIM, DIM)

    wf = np.ascontiguousarray(
        Wr[:, :128].reshape(COUT, 128, 27).transpose(1, 2, 0)
    ).astype(bf16)

    # tail weights: row b*16+c, phase i gets W[co, 128+c, S[b]+T[i]] if that
    # tap is assigned to (b, i), else 0 (covers each tap exactly once)
    assign = _tail_assign()
    wt = np.zeros((128, NTAIL, COUT), np.float32)
    tailW = Wr[:, 128:144]  # [co, c, dx, dy, dz]
    for tap, (b, i) in assign.items():
        dx, dy, dz = tap
        r = b * 16
        wt[r: r + 16, i, :] = tailW[:, :, dx, dy, dz].T
    wt = wt.astype(fp8)

    return [{"x": xb[b], "xt": xt[b], "wf": wf, "wt": wt}
            for b in range(N_CORES)]


def kernel(x, W, _trace=False):
    nc = _get_nc()
    in_maps = _prep_inputs(np.asarray(x), np.asarray(W))
    res = None
    for attempt in range(3):
        try:
            res = run_bass_kernel_spmd(nc, in_maps, list(range(N_CORES)),
                                       trace=_trace)
            break
        except Exception:
            # rare transient NRT_EXEC_UNIT_UNRECOVERABLE flakes; retry
            if attempt == 2:
                raise
            import time as _time
            _time.sleep(2.0)
    full = np.empty((N_CORES, COUT, ODIM, ODIM, ODIM), np.float32)
    for b in range(N_CORES):
        o = np.asarray(res.results[b]["out"]).astype(np.float32)
        nb = 0
        g0 = 0
        for gsz in GROUP_SIZES:
            for bi in range(gsz // 4):
                for j in range(4):
                    ox, h = CHUNKS[g0 + 4 * bi + j]
                    full[b, :, ox, 15 * h: 15 * h + 15, :] = (
                        o[32 * j: 32 * j + 32, nb].reshape(COUT, 15, 30))
                nb += 1
            g0 += gsz
    if _trace:
        return full, res
    return full
